# revision 12
# baseline (speedup 1.0000x reference)
"""Trainium2 Bass kernel for nn_BAGDnet (gnn_message_passing).

Computation (per measurement m):
    T = tKF[meas_kf[m]]          # 4x4 pose
    p = tMP[meas_mp[m]]          # 3d map point
    pts = T[:3] @ [p, 1]
    out[m] = (pts0/pts2*FX + CX, pts1/pts2*FY + CY)

idxKF / idxMP are sorted unique arange id tables, so searchsorted(idx, meas)
== meas and measurement ids index the tables directly.

Sharding strategy (data-parallel over M per the hint): 2M measurements split
across 8 cores. Per core, measurements are grouped by pose into fixed-size
cells (S=8 slots, one pose per cell, poses spanning multiple cells get their
table row duplicated), laid out as 128 partitions x 256 cells. The pose rows
are pre-projected on host into A = [FX*T0+CX*T2; FY*T1+CY*T2; T2] so the
device computes out = (A0.h/A2.h, A1.h/A2.h) with no epilogue add.

On device the pose row for a cell is never materialized per measurement:
the multiply reads the 12-value row straight from a tiny per-partition table
through a stride-0 broadcast access pattern. This cuts HBM traffic from
68 B/meas (gathered-pose streaming) to ~15 B/meas (fp16 h-vector + fp16 out
+ table), turning the kernel from DMA-bound into engine-balanced:
  DVE    : product m = A (*) h   (fp16 TensorTensor, 2x mode) + first adds
  GPSIMD : second adds (scalar_tensor_tensor) + perspective divide
Host gathers the points into cell order (id->row is identity here), and
un-permutes the fp16 device output back to measurement order in f32.
"""

import numpy as np

M = 2_000_000
N_KF = 2_000
N_MP = 200_000
N_CORES = 8
MC = M // N_CORES          # 250_000 measurements per core
P = 128
S = 8                      # slots per cell (one pose per cell)
CH = 256                   # cells per partition
SLOTS = CH * S             # 2048 slots per partition
TOT = P * SLOTS            # 262144 slots per core (~4.9% padding)
# ramped slab schedule (slots per slab, multiples of S): small head shortens
# pipeline fill, small tail shortens drain
SLABS = [64, 160, 288, 352, 352, 352, 320, 160]
assert sum(SLABS) == SLOTS and all(s % S == 0 for s in SLABS)
FX = 320.0
FY = 320.0
CX = 320.0
CY = 240.0

_CACHE = {}


def _build():
    import concourse.bacc as bacc
    import concourse.mybir as mybir
    import concourse.tile as tile

    f16 = mybir.dt.float16
    f32 = mybir.dt.float32
    mult, add = mybir.AluOpType.mult, mybir.AluOpType.add

    nc = bacc.Bacc("TRN2", target_bir_lowering=False, debug=False)
    hp = nc.dram_tensor("hp", [P, SLOTS * 4], f16, kind="ExternalInput")
    tb = nc.dram_tensor("tb", [P, CH * 12], f16, kind="ExternalInput")
    ot = nc.dram_tensor("ot", [P, SLOTS * 2], f16, kind="ExternalOutput")

    with tile.TileContext(nc) as tc:
        with tc.tile_pool(name="hpool", bufs=3) as hpool, \
             tc.tile_pool(name="tpool", bufs=3) as tpool, \
             tc.tile_pool(name="mpool", bufs=3) as mpool, \
             tc.tile_pool(name="spool", bufs=3) as spool, \
             tc.tile_pool(name="apool", bufs=3) as apool, \
             tc.tile_pool(name="opool", bufs=3) as opool:
            so = 0
            for o, sls in enumerate(SLABS):
                chs = sls // S
                co = so // S
                ld_a = nc.sync if o % 2 == 0 else nc.scalar
                ld_b = nc.scalar if o % 2 == 0 else nc.sync
                ht = hpool.tile([P, sls * 4], f16, tag="ht")
                tt = tpool.tile([P, chs * 12], f16, tag="tt")
                ld_a.dma_start(out=ht[:], in_=hp.ap()[:, so * 4:(so + sls) * 4])
                ld_b.dma_start(out=tt[:], in_=tb.ap()[:, co * 12:(co + chs) * 12])
                # m[p, cell, s, i, j] = A[p, cell, i, j] * h[p, cell, s, j]
                m = mpool.tile([P, sls * 12], f16, tag="m")
                h_b = ht[:].rearrange("p (seg s o j) -> p seg s o j",
                                      seg=chs, s=S, o=1, j=4) \
                           .to_broadcast([P, chs, S, 3, 4])
                a_b = tt[:].rearrange("p (seg o i j) -> p seg o i j",
                                      seg=chs, o=1, i=3, j=4) \
                           .to_broadcast([P, chs, S, 3, 4])
                m_v = m[:].rearrange("p (seg s i j) -> p seg s i j",
                                     seg=chs, s=S, i=3, j=4)
                nc.vector.tensor_tensor(out=m_v, in0=h_b, in1=a_b, op=mult)
                # s1[p, sl, i, k] = m[.., i, k] + m[.., i, k+2]   (DVE, 2x)
                mv = m[:].rearrange("p (sl i j) -> p sl i j", i=3, j=4)
                s1 = spool.tile([P, sls * 6], f16, tag="s1")
                s1v = s1[:].rearrange("p (sl i k) -> p sl i k", i=3, k=2)
                nc.vector.tensor_tensor(out=s1v, in0=mv[:, :, :, 0:2],
                                        in1=mv[:, :, :, 2:4], op=add)
                # z[p, sl]      = s1[.., 2, 0] + s1[.., 2, 1]     (gpsimd, f32)
                # a01[p, sl, c] = s1[.., c, 0] + s1[.., c, 1]     (gpsimd, fp16)
                # z is issued first so the DVE reciprocal overlaps a01
                z = apool.tile([P, sls], f32, tag="z")
                nc.gpsimd.tensor_tensor(out=z[:], in0=s1v[:, :, 2, 0],
                                        in1=s1v[:, :, 2, 1], op=add)
                a01 = apool.tile([P, sls * 2], f16, tag="a01")
                a01v = a01[:].rearrange("p (sl c) -> p sl c", c=2)
                nc.gpsimd.tensor_tensor(out=a01v, in0=s1v[:, :, 0:2, 0],
                                        in1=s1v[:, :, 0:2, 1], op=add)
                # rz = 1/z (DVE, f32), out = a01 * rz (gpsimd)
                rz = apool.tile([P, sls], f32, tag="rz")
                nc.vector.reciprocal_approx_fast(out=rz[:], in_=z[:])
                otile = opool.tile([P, sls * 2], f16, tag="ot")
                ov = otile[:].rearrange("p (sl c) -> p sl c", c=2)
                rzb = rz[:].rearrange("p (sl o) -> p sl o", o=1) \
                           .to_broadcast([P, sls, 2])
                nc.gpsimd.tensor_tensor(out=ov, in0=a01v, in1=rzb,
                                        op=mybir.AluOpType.mult)
                ld_b.dma_start(out=ot.ap()[:, so * 2:(so + sls) * 2],
                               in_=otile[:])
                so += sls
    nc.compile()
    return nc


def get_nc():
    if "nc" not in _CACHE:
        _CACHE["nc"] = _build()
    return _CACHE["nc"]


def make_in_maps(tMP, tKF, kf_rows, mp_rows):
    """Pack measurements into pose-cells; returns per-core inputs + slot maps."""
    T = np.asarray(tKF, dtype=np.float32)
    A = np.empty((N_KF, 3, 4), np.float32)
    A[:, 0] = FX * T[:, 0] + CX * T[:, 2]
    A[:, 1] = FY * T[:, 1] + CY * T[:, 2]
    A[:, 2] = T[:, 2]
    A12 = A.reshape(N_KF, 12).astype(np.float16)
    empty_row = np.zeros(12, np.float16)
    empty_row[11] = 1.0        # a2 = 1 for padding cells -> out = 0, no NaN
    tMP = np.asarray(tMP, dtype=np.float32)
    homo = np.concatenate([tMP, np.ones((N_MP, 1), np.float32)], axis=1) \
             .astype(np.float16)
    in_maps = []
    slot_maps = []
    for c in range(N_CORES):
        kf = kf_rows[c * MC:(c + 1) * MC]
        mp = mp_rows[c * MC:(c + 1) * MC]
        counts = np.bincount(kf, minlength=N_KF)
        ncells_k = -(-counts // S)
        cell_off = np.concatenate([[0], np.cumsum(ncells_k)])
        ncells = int(cell_off[-1])
        assert ncells <= P * CH, f"cell overflow: {ncells} > {P * CH}"
        order = np.argsort(kf, kind="stable")
        kfs = kf[order]
        starts = np.concatenate([[0], np.cumsum(counts)])
        j = np.arange(MC, dtype=np.int64) - starts[kfs]
        slot = (cell_off[kfs] + j // S) * S + (j % S)    # flat in [0, TOT)
        hpa = np.zeros((TOT, 4), np.float16)
        hpa[:, 3] = 1.0
        hpa[slot] = homo[mp[order]]
        kcell = np.repeat(np.arange(N_KF), ncells_k)
        tbl = np.empty((P * CH, 12), np.float16)
        tbl[:ncells] = A12[kcell]
        tbl[ncells:] = empty_row
        in_maps.append({"hp": hpa.reshape(P, SLOTS * 4),
                        "tb": tbl.reshape(P, CH * 12)})
        slot_maps.append((order, slot))
    return in_maps, slot_maps


def assemble(results, slot_maps):
    outs = []
    for c in range(N_CORES):
        o = np.asarray(results[c]["ot"]).reshape(TOT, 2)
        order, slot = slot_maps[c]
        r = np.empty((MC, 2), np.float32)
        r[order] = o[slot].astype(np.float32)
        outs.append(r)
    return np.concatenate(outs, axis=0)


def kernel(tMP, tKF, idxKF, idxMP, meas_kf, meas_mp):
    import time

    from concourse.bass_utils import run_bass_kernel_spmd

    nc = get_nc()
    # id -> row resolution (identity for sorted arange id tables)
    kf_rows = np.searchsorted(np.asarray(idxKF), np.asarray(meas_kf)).astype(np.int64)
    mp_rows = np.searchsorted(np.asarray(idxMP), np.asarray(meas_mp)).astype(np.int64)
    in_maps, slot_maps = make_in_maps(np.asarray(tMP), np.asarray(tKF),
                                      kf_rows, mp_rows)
    try:
        res = run_bass_kernel_spmd(nc, in_maps, core_ids=list(range(N_CORES)))
    except Exception:
        # transient NRT exec-unit errors have been observed when a previous
        # process was still draining the cores; one retry recovers them
        time.sleep(2.0)
        res = run_bass_kernel_spmd(nc, in_maps, core_ids=list(range(N_CORES)))
    return assemble(res.results, slot_maps)
